# revision 21
# baseline (speedup 1.0000x reference)
"""Trainium2 Bass kernel for AlignToZ Wigner-D (l=0..4, DIM=25).

Strategy: every nonzero entry of the 25x25 block-diagonal output D is a
polynomial in (ct, st, cos m*phi, sin m*phi).  With st^2 -> 1-ct^2 the
whole map reduces to D[n, o] = sum_k Feat[n, k] * W[k, o] with K=81
monomial features and 164 block columns (l=1..4; l=0 col is constant 1).

Per core (data-parallel over N): build Feat with wide DVE/ACT ops
(points laid out [128 partitions x F free]), PE-transpose 128-point
chunks of Feat to [K, 128], one fp32 matmul per chunk against the
constant W -> PSUM [128, 164], scatter-copy into persistent pre-zeroed
625-wide out tiles, DMA contiguous rows to DRAM.
"""
import math
import numpy as np
from collections import defaultdict
from contextlib import ExitStack

# ----------------------------------------------------------------------------
# Symbolic table generation (monomials: ct^a * st^b * {cos,sin}(m phi))
# ----------------------------------------------------------------------------

def _p_terms(i, a, b, l):
    lp = l - 1
    n = 2 * lp + 1
    row = i + 1
    if b == l:
        return [(1.0, row * 3 + 2, (a + lp) * n + 2 * lp),
                (-1.0, row * 3 + 0, (a + lp) * n + 0)]
    if b == -l:
        return [(1.0, row * 3 + 2, (a + lp) * n + 0),
                (1.0, row * 3 + 0, (a + lp) * n + 2 * lp)]
    return [(1.0, row * 3 + 1, (a + lp) * n + (b + lp))]


def _ru_tables(l):
    out_idx, rij, ab, coef = [], [], [], []
    n = 2 * l + 1
    for m in range(-l, l + 1):
        for mp in range(-l, l + 1):
            denom = (2 * l) * (2 * l - 1) if abs(mp) == l else (l + mp) * (l - mp)
            d0 = 1.0 if m == 0 else 0.0
            u = math.sqrt((l + m) * (l - m) / denom)
            v = 0.5 * math.sqrt((1 + d0) * (l + abs(m) - 1) * (l + abs(m)) / denom) * (1 - 2 * d0)
            w = -0.5 * math.sqrt(max(l - abs(m) - 1, 0) * (l - abs(m)) / denom) * (1 - d0)
            terms = []
            if u != 0.0:
                terms += [(u * c, i, j) for c, i, j in _p_terms(0, m, mp, l)]
            if v != 0.0:
                if m == 0:
                    terms += [(v * c, i, j) for c, i, j in _p_terms(1, 1, mp, l)]
                    terms += [(v * c, i, j) for c, i, j in _p_terms(-1, -1, mp, l)]
                elif m > 0:
                    s = math.sqrt(2.0) if m == 1 else 1.0
                    terms += [(v * s * c, i, j) for c, i, j in _p_terms(1, m - 1, mp, l)]
                    if m != 1:
                        terms += [(-v * c, i, j) for c, i, j in _p_terms(-1, -m + 1, mp, l)]
                else:
                    if m != -1:
                        terms += [(v * c, i, j) for c, i, j in _p_terms(1, m + 1, mp, l)]
                    s = math.sqrt(2.0) if m == -1 else 1.0
                    terms += [(v * s * c, i, j) for c, i, j in _p_terms(-1, -m - 1, mp, l)]
            if w != 0.0:
                if m > 0:
                    terms += [(w * c, i, j) for c, i, j in _p_terms(1, m + 1, mp, l)]
                    terms += [(w * c, i, j) for c, i, j in _p_terms(-1, -m - 1, mp, l)]
                else:
                    terms += [(w * c, i, j) for c, i, j in _p_terms(1, m - 1, mp, l)]
                    terms += [(-w * c, i, j) for c, i, j in _p_terms(-1, -m + 1, mp, l)]
            o = (m + l) * n + (mp + l)
            for c, i, j in terms:
                out_idx.append(o); rij.append(i); ab.append(j); coef.append(c)
    return out_idx, rij, ab, coef


def _mono_mul(k1, k2):
    a = k1[0] + k2[0]
    b = k1[1] + k2[1]
    t1, m1 = k1[2], k1[3]
    t2, m2 = k2[2], k2[3]
    if t1 == 'c' and t2 == 'c':
        tt = [(0.5, 'c', m1 + m2), (0.5, 'c', abs(m1 - m2))]
    elif t1 == 's' and t2 == 's':
        tt = [(-0.5, 'c', m1 + m2), (0.5, 'c', abs(m1 - m2))]
    else:
        ms, mc = (m1, m2) if t1 == 's' else (m2, m1)
        d = ms - mc
        tt = [(0.5, 's', ms + mc)]
        if d > 0:
            tt.append((0.5, 's', d))
        elif d < 0:
            tt.append((-0.5, 's', -d))
    out = []
    for cf, t, m in tt:
        if t == 's' and m == 0:
            continue
        out.append((cf, (a, b, t, m)))
    return out


def _reduce_st(poly):
    changed = True
    while changed:
        changed = False
        for k in list(poly.keys()):
            a, b, t, m = k
            if b >= 2:
                c = poly.pop(k)
                for cf2, anew in ((1.0, a), (-1.0, a + 2)):
                    k2 = (anew, b - 2, t, m)
                    poly[k2] = poly.get(k2, 0.0) + cf2 * c
                changed = True
    return {k: v for k, v in poly.items() if abs(v) > 1e-12}


def _poly_mul(p1, p2):
    out = defaultdict(float)
    for k1, c1 in p1.items():
        for k2, c2 in p2.items():
            for cf, k in _mono_mul(k1, k2):
                out[k] += c1 * c2 * cf
    return _reduce_st(dict(out))


def _build_symbolic_blocks():
    ct = (1, 0, 'c', 0); st = (0, 1, 'c', 0)
    cp = (0, 0, 'c', 1); sp = (0, 0, 's', 1)
    r1 = [
        {cp: 1.0}, {}, {sp: -1.0},
        {(0, 1, 's', 1): 1.0}, {ct: 1.0}, {(0, 1, 'c', 1): 1.0},
        {(1, 0, 's', 1): 1.0}, {st: -1.0}, {(1, 0, 'c', 1): 1.0},
    ]
    blocks = {1: r1}
    for l in range(2, 5):
        oi, rij, ab, cf = _ru_tables(l)
        n = 2 * l + 1
        cur = [defaultdict(float) for _ in range(n * n)]
        prev = blocks[l - 1]
        for o, i, j, c in zip(oi, rij, ab, cf):
            p = _poly_mul(r1[i], prev[j])
            for k, v in p.items():
                cur[o][k] += c * v
        blocks[l] = [_reduce_st(dict(d)) for d in cur]
    return blocks


def build_tables():
    """Returns (monos, W) with W shape [K, 164] fp32. Output cols are the
    flattened l=1..4 block entries in order (l, r, c)."""
    blocks = _build_symbolic_blocks()
    monos = set()
    for l in range(1, 5):
        for poly in blocks[l]:
            monos |= set(poly.keys())
    monos = sorted(monos)
    midx = {m: i for i, m in enumerate(monos)}
    ncols = sum((2 * l + 1) ** 2 for l in range(1, 5))
    W = np.zeros((len(monos), ncols), dtype=np.float64)
    col = 0
    for l in range(1, 5):
        n = 2 * l + 1
        for o in range(n * n):
            for k, v in blocks[l][o].items():
                W[midx[k], col] = v
            col += 1
    return monos, W.astype(np.float32)


MONOS, W_TABLE = build_tables()
K = len(MONOS)          # 81
NOUT = W_TABLE.shape[1]  # 164
assert K <= 128 and NOUT == 164
# fp16 hi/lo split of W: 3-term split GEMM keeps absmax err ~1e-6 while
# halving PE bytes vs the 2-pass fp32 matmul
_WHI = W_TABLE.astype(np.float16)
_WLO = (W_TABLE - _WHI.astype(np.float32)).astype(np.float16)
W_SPLIT = np.concatenate([_WHI, _WLO], axis=1)  # [K, 2*NOUT] fp16

# ----------------------------------------------------------------------------
# Bass program
# ----------------------------------------------------------------------------
N_TOTAL = 200000
N_CORES = 8
N_PER = N_TOTAL // N_CORES  # 25000
DIM = 25
ROW = DIM * DIM  # 625
BCH = 3          # chunks (of 128 points) per batch
NSLOT = 6        # persistent out-tile rotation depth
_OFF = [0, 1, 4, 9, 16]  # block row/col offset per l


def _feat_program(nc, Feat, XYZ, Fs, scratch_pool, dt):
    """Emit wide ops computing all K monomial columns into Feat
    (layout [128, Fs*K], col k of f-slot f at f*K + k)."""
    import concourse.mybir as mybir
    Alu = mybir.AluOpType

    fv = Feat.rearrange("p (f k) -> p k f", k=K)

    def slot(key):
        return fv[:, MONOS.index(key), :]

    xyzv = XYZ.rearrange("p (f k) -> p k f", k=3)
    x, y, z = xyzv[:, 0, :], xyzv[:, 1, :], xyzv[:, 2, :]

    sc = [scratch_pool.tile([128, Fs], dt, name=f"sc{i}", tag=f"sc{i}")
          for i in range(5)]
    s0, s1, s2, s3, s4 = [t[:] for t in sc]

    act, dve = nc.scalar, nc.vector

    def rsqrt_nr(dst, a, tmp):
        """dst = 1/sqrt(a), ACT-sqrt seed + one Newton step (~1 ulp)."""
        act.sqrt(dst, a)
        dve.reciprocal(dst, dst)            # y0 (rel err ~7e-6 from LUT sqrt)
        dve.tensor_mul(tmp, dst, dst)       # y0^2
        dve.tensor_mul(tmp, a, tmp)         # a*y0^2
        dve.tensor_scalar(tmp, tmp, -0.5, 1.5, Alu.mult, Alu.add)
        dve.tensor_mul(dst, dst, tmp)       # y1 = y0*(1.5 - 0.5*a*y0^2)

    # prologue: ct, st, c1, s1
    act.square(s0, x)
    act.square(s1, y)
    dve.tensor_add(s0, s0, s1)          # rxy2
    act.square(s1, z)
    dve.tensor_add(s1, s0, s1)          # r2
    dve.tensor_scalar_max(s0, s0, 1e-37)
    dve.tensor_scalar_max(s1, s1, 1e-28)
    rsqrt_nr(s2, s1, s3)                # rinv  = 1/max(r, 1e-14)
    rsqrt_nr(s4, s0, s3)                # rxyinv
    dve.tensor_mul(s3, z, s2)           # ct raw
    ct = slot((1, 0, 'c', 0))
    dve.tensor_scalar(ct, s3, -1.0, 1.0, Alu.max, Alu.min)
    st = slot((0, 1, 'c', 0))
    dve.tensor_mul(s0, s0, s4)          # rxy = rxy2 * rxyinv
    dve.tensor_mul(st, s0, s2)          # st = rxy * rinv
    c1 = slot((0, 0, 'c', 1))
    s1t = slot((0, 0, 's', 1))
    dve.tensor_mul(c1, x, s4)
    dve.tensor_mul(s1t, y, s4)
    # chebyshev trig: c2=2c1^2-1, s2=2s1c1, cm=2c1*c(m-1)-c(m-2), ...
    trig = {('c', 1): c1, ('s', 1): s1t}
    trig[('c', 0)] = None  # constant one
    for m in range(2, 5):
        cm = slot((0, 0, 'c', m))
        sm = slot((0, 0, 's', m))
        if m == 2:
            dve.scalar_tensor_tensor(s3, c1, 2.0, c1, Alu.mult, Alu.mult)
            dve.tensor_scalar_add(cm, s3, -1.0)
            dve.scalar_tensor_tensor(sm, s1t, 2.0, c1, Alu.mult, Alu.mult)
        else:
            dve.scalar_tensor_tensor(s3, trig[('c', m - 1)], 2.0, c1, Alu.mult, Alu.mult)
            dve.tensor_sub(cm, s3, trig[('c', m - 2)])
            dve.scalar_tensor_tensor(s3, trig[('s', m - 1)], 2.0, c1, Alu.mult, Alu.mult)
            dve.tensor_sub(sm, s3, trig[('s', m - 2)])
        trig[('c', m)] = cm
        trig[('s', m)] = sm
    # ct powers (present as monomials (a,0,'c',0))
    ctp = {1: ct}
    for a in range(2, 5):
        key = (a, 0, 'c', 0)
        dst = slot(key) if key in MONOS else None
        assert dst is not None, key
        h = a // 2
        dve.tensor_mul(dst, ctp[h], ctp[a - h])
        ctp[a] = dst
    # constant-one column
    kone = MONOS.index((0, 0, 'c', 0))
    nc.vector.memset(fv[:, kone, :], 1.0)
    # st*trig columns (0,1,t,m)
    stt = {('c', 0): st}
    for (t, m) in [(t, m) for t in 'cs' for m in range(1, 5)]:
        key = (0, 1, t, m)
        if key in MONOS:
            dve.tensor_mul(slot(key), st, trig[(t, m)])
            stt[(t, m)] = slot(key)
    # remaining monomials: a>=1 times trig or st*trig
    done = {(0, 0, 'c', 0), (1, 0, 'c', 0), (0, 1, 'c', 0)}
    done |= {(0, 0, t, m) for t in 'cs' for m in range(1, 5)}
    done |= {(a, 0, 'c', 0) for a in range(2, 5)}
    done |= {(0, 1, t, m) for t in 'cs' for m in range(1, 5)}
    for key in MONOS:
        if key in done:
            continue
        a, b, t, m = key
        assert a >= 1, key
        base = stt[(t, m)] if b else trig[(t, m)]
        assert base is not None
        dve.tensor_mul(slot(key), ctp[a], base)


def build_program(n_pad, hw=True):
    """n_pad must be a multiple of 128 (host pads input/slices output).
    hw=False skips the walrus wait-limit postprocessing (CoreSim can't
    execute the synthetic NOPs)."""
    import concourse.bass as bass
    import concourse.tile as tile
    import concourse.mybir as mybir
    from concourse.masks import make_identity

    dt = mybir.dt.float32
    assert n_pad % 128 == 0
    Fs = n_pad // 128                  # f-slots (all full)

    nc = bass.Bass()
    f16 = mybir.dt.float16
    xyz_d = nc.declare_dram_parameter("xyz", [n_pad, 3], dt, isOutput=False)
    wc_d = nc.declare_dram_parameter("wc", [K, 2 * NOUT], f16, isOutput=False)
    out_d = nc.declare_dram_parameter("out", [n_pad, ROW], dt, isOutput=True)

    join_sems = {}
    with tile.TileContext(nc) as tc, ExitStack() as ctx:
        join_sems['DVE'] = nc.alloc_semaphore("join_dve")
        join_sems['Activation'] = nc.alloc_semaphore("join_act")
        consts = ctx.enter_context(tc.tile_pool(name="consts", bufs=1))
        scratch = ctx.enter_context(tc.tile_pool(name="scratch", bufs=1))
        featb_pool = ctx.enter_context(tc.tile_pool(name="featb", bufs=4))
        psum_t = ctx.enter_context(tc.tile_pool(name="psumT", bufs=3, space="PSUM"))
        psum_o = ctx.enter_context(tc.tile_pool(name="psumO", bufs=3, space="PSUM"))

        ident = consts.tile([128, 128], dt, tag="ident")
        make_identity(nc, ident[:])
        wc = consts.tile([K, 2 * NOUT], f16, tag="wc")
        nc.sync.dma_start(wc[:], wc_d[:, :])

        XYZ = consts.tile([128, Fs * 3], dt, tag="xyzt")
        nc.sync.dma_start(
            XYZ[:].rearrange("p (f k) -> p f k", k=3),
            xyz_d[:, :].rearrange("(f p) k -> p f k", p=128))

        Feat = consts.tile([128, Fs * K], dt, tag="feat")
        _feat_program(nc, Feat[:], XYZ[:], Fs, scratch, dt)

        # persistent out tiles: zeros + l0 ones written once.  Each slot is
        # written by exactly one compute engine (DVE for even slots, ACT for
        # odd) so downstream instructions never need >2 semaphore waits.
        outs = []
        for i in range(NSLOT):
            t = consts.tile([128, BCH * ROW], dt, name=f"out{i}", tag=f"out{i}")
            v = t[:].rearrange("p (c r q) -> p c r q", r=DIM, q=DIM)
            ones = v[:, :, 0:1, 0:1]
            if i % 2 == 0:
                nc.vector.memset(t[:], 0.0)
                nc.vector.memset(ones, 1.0)
            else:
                nc.scalar.memzero(t[:])
                nc.scalar.add(ones, ones, 1.0)
            outs.append(t)

        featv = Feat[:].rearrange("p (f k) -> p f k", k=K)
        nbatch = (Fs + BCH - 1) // BCH
        for bi in range(nbatch):
            f0 = bi * BCH
            nch = min(BCH, Fs - f0)
            pt = psum_t.tile([K, 128 * BCH], dt, tag="pt")
            for c in range(nch):
                nc.tensor.transpose(pt[:, 128 * c:128 * (c + 1)],
                                    featv[:, f0 + c, :], ident[:])
            fhi = featb_pool.tile([K, 128 * BCH], f16, tag="fhi")
            flo = featb_pool.tile([K, 128 * BCH], f16, tag="flo")
            sl = slice(0, 128 * nch)
            nc.scalar.copy(fhi[:, sl], pt[:, sl])
            nc.vector.scalar_tensor_tensor(
                flo[:, sl], pt[:, sl], 1.0, fhi[:, sl],
                mybir.AluOpType.mult, mybir.AluOpType.subtract)
            po = psum_o.tile([128, NOUT * BCH], dt, tag="po")
            whi, wlo = wc[:, 0:NOUT], wc[:, NOUT:2 * NOUT]
            for c in range(nch):
                cs = slice(128 * c, 128 * (c + 1))
                os_ = slice(NOUT * c, NOUT * (c + 1))
                nc.tensor.matmul(po[:, os_], fhi[:, cs], whi,
                                 start=True, stop=False)
                nc.tensor.matmul(po[:, os_], fhi[:, cs], wlo,
                                 start=False, stop=False)
                nc.tensor.matmul(po[:, os_], flo[:, cs], whi,
                                 start=False, stop=True)
            # scatter into out tile
            ot = outs[bi % NSLOT]
            ov = ot[:].rearrange("p (c r q) -> p c r q", r=DIM, q=DIM)
            pov = po[:].rearrange("m (c e) -> m c e", c=BCH)
            base = 0
            for l in range(1, 5):
                n = 2 * l + 1
                off = _OFF[l]
                src = pov[:, 0:nch, base:base + n * n].rearrange(
                    "m c (r q) -> m c r q", r=n)
                dst = ov[:, 0:nch, off:off + n, off:off + n]
                if l >= 3:
                    nc.vector.tensor_copy(dst, src)
                else:
                    nc.scalar.copy(dst, src)
                base += n * n
            # DMA out
            rows0 = f0 * 128
            nc.sync.dma_start(
                out_d[rows0:rows0 + 128 * nch, :].rearrange(
                    "(c p) e -> p c e", p=128),
                ot[:, 0:nch * ROW].rearrange("p (c e) -> p c e", e=ROW))
    nc.finalize()
    if hw:
        fix_dma_waits(nc, join_sems)
        split_sync_waits(nc)
    return nc


_ENG_PREFIX = {'DVE': 'DVE', 'Activation': 'Activation', 'Pool': 'Pool',
               'PE': 'PE', 'SP': 'SP'}


def fix_dma_waits(nc, join_sems):
    """DMA descriptors here accept a single wait.  For DMAs with >1 wait,
    host a NOP chain on the engine whose own semaphore is among the waits
    (queue order makes that wait implicit), absorb the remaining waits there,
    and signal a dedicated join semaphore the DMA waits on instead."""
    import concourse.mybir as mybir
    ET = mybir.EngineType
    eng_map = {'DVE': ET.DVE, 'Activation': ET.Activation, 'Pool': ET.Pool,
               'PE': ET.PE, 'SP': ET.SP}
    counters = {e: 0 for e in join_sems}
    nfix = 0
    for f in nc.m.functions:
        for bb in f.blocks:
            insl = bb.instructions
            i = 0
            while i < len(insl):
                ins = insl[i]
                si = ins.sync_info
                if (ins.opcode in ('DMACopy', 'DMA') and si is not None
                        and len(si.on_wait) > 1):
                    waits = list(si.on_wait)
                    host = None
                    rest = []
                    for w in waits:
                        pfx = w.ant_name.split('_')[0]
                        if host is None and pfx in eng_map and pfx in join_sems:
                            host = pfx  # implied by queue order; dropped
                        else:
                            rest.append(w)
                    assert host is not None, \
                        f"DMA {ins.name} waits {[w.ant_name for w in waits]}"
                    js = join_sems[host]
                    for j, w in enumerate(rest):
                        nop = mybir.InstNoOp(name=f"{ins.name}-dj{j}")
                        nop.engine = eng_map[host]
                        upd = []
                        if j == len(rest) - 1:
                            counters[host] += 1
                            upd = [mybir.SyncUpdate(
                                ant_name=js.name, id=js.num,
                                sync_type='semaphore', update_mode='sem-inc',
                                update_value=1)]
                        nop.sync_info = mybir.SyncInfo(on_wait=[w], on_update=upd)
                        insl.insert(i, nop)
                        i += 1
                    ins.sync_info = mybir.SyncInfo(
                        on_wait=[mybir.SyncWait(
                            ant_name=js.name, id=js.num, sync_type='semaphore',
                            wait_mode='sem-ge-imm', wait_value=counters[host])],
                        on_update=list(si.on_update))
                    nfix += 1
                i += 1
    return nfix


def split_sync_waits(nc, keep=1):
    """The walrus build here accepts at most one embedded semaphore wait per
    engine instruction: hoist extra waits onto single-wait NOPs in front (same
    engine queue, so in-order semantics are identical)."""
    import concourse.mybir as mybir
    nsplit = 0
    for f in nc.m.functions:
        for bb in f.blocks:
            insl = bb.instructions
            i = 0
            while i < len(insl):
                ins = insl[i]
                si = ins.sync_info
                if (si is not None and len(si.on_wait) > keep
                        and ins.opcode not in ('DMACopy', 'DMA')):
                    waits = list(si.on_wait)
                    head, tail = waits[:-keep], waits[-keep:]
                    for j, w in enumerate(head):
                        nop = mybir.InstNoOp(name=f"{ins.name}-sw{j}")
                        nop.engine = ins.engine
                        nop.sync_info = mybir.SyncInfo(on_wait=[w], on_update=[])
                        insl.insert(i, nop)
                        i += 1
                        nsplit += 1
                    ins.sync_info = mybir.SyncInfo(
                        on_wait=tail, on_update=list(si.on_update))
                i += 1
    return nsplit


# ----------------------------------------------------------------------------
# Entry point
# ----------------------------------------------------------------------------
_CACHE = {}


def kernel(xyz: np.ndarray) -> np.ndarray:
    from concourse.bass_utils import run_bass_kernel_spmd

    assert xyz.shape == (N_TOTAL, 3)
    n_pad = ((N_PER + 127) // 128) * 128
    if "nc" not in _CACHE:
        _CACHE["nc"] = build_program(n_pad)
    nc = _CACHE["nc"]

    xyz = np.ascontiguousarray(xyz, dtype=np.float32)
    in_maps = []
    for c in range(N_CORES):
        shard = xyz[c * N_PER:(c + 1) * N_PER]
        if n_pad != N_PER:
            shard = np.concatenate(
                [shard, np.repeat(shard[-1:], n_pad - N_PER, axis=0)], axis=0)
        in_maps.append({"xyz": np.ascontiguousarray(shard), "wc": W_SPLIT})
    res = run_bass_kernel_spmd(nc, in_maps, list(range(N_CORES)))
    outs = [r["out"][:N_PER] for r in res.results]
    full = np.concatenate(outs, axis=0).reshape(N_TOTAL, DIM, DIM)
    return full


if __name__ == "__main__":
    rng = np.random.default_rng(0)
    pts = rng.normal(size=(N_TOTAL, 3)).astype(np.float32)
    out = kernel(pts)
    print(out.shape, out.dtype)


# revision 22
# speedup vs baseline: 1.1634x; 1.1634x over previous
"""Trainium2 Bass kernel for AlignToZ Wigner-D (l=0..4, DIM=25).

Strategy: every nonzero entry of the 25x25 block-diagonal output D is a
polynomial in (ct, st, cos m*phi, sin m*phi).  With st^2 -> 1-ct^2 the
whole map reduces to D[n, o] = sum_k Feat[n, k] * W[k, o] with K=81
monomial features and 164 block columns (l=1..4; l=0 col is constant 1).

Per core (data-parallel over N): build Feat with wide DVE/ACT ops
(points laid out [128 partitions x F free]), PE-transpose 128-point
chunks of Feat to [K, 128], one fp32 matmul per chunk against the
constant W -> PSUM [128, 164], scatter-copy into persistent pre-zeroed
625-wide out tiles, DMA contiguous rows to DRAM.
"""
import math
import numpy as np
from collections import defaultdict
from contextlib import ExitStack

# ----------------------------------------------------------------------------
# Symbolic table generation (monomials: ct^a * st^b * {cos,sin}(m phi))
# ----------------------------------------------------------------------------

def _p_terms(i, a, b, l):
    lp = l - 1
    n = 2 * lp + 1
    row = i + 1
    if b == l:
        return [(1.0, row * 3 + 2, (a + lp) * n + 2 * lp),
                (-1.0, row * 3 + 0, (a + lp) * n + 0)]
    if b == -l:
        return [(1.0, row * 3 + 2, (a + lp) * n + 0),
                (1.0, row * 3 + 0, (a + lp) * n + 2 * lp)]
    return [(1.0, row * 3 + 1, (a + lp) * n + (b + lp))]


def _ru_tables(l):
    out_idx, rij, ab, coef = [], [], [], []
    n = 2 * l + 1
    for m in range(-l, l + 1):
        for mp in range(-l, l + 1):
            denom = (2 * l) * (2 * l - 1) if abs(mp) == l else (l + mp) * (l - mp)
            d0 = 1.0 if m == 0 else 0.0
            u = math.sqrt((l + m) * (l - m) / denom)
            v = 0.5 * math.sqrt((1 + d0) * (l + abs(m) - 1) * (l + abs(m)) / denom) * (1 - 2 * d0)
            w = -0.5 * math.sqrt(max(l - abs(m) - 1, 0) * (l - abs(m)) / denom) * (1 - d0)
            terms = []
            if u != 0.0:
                terms += [(u * c, i, j) for c, i, j in _p_terms(0, m, mp, l)]
            if v != 0.0:
                if m == 0:
                    terms += [(v * c, i, j) for c, i, j in _p_terms(1, 1, mp, l)]
                    terms += [(v * c, i, j) for c, i, j in _p_terms(-1, -1, mp, l)]
                elif m > 0:
                    s = math.sqrt(2.0) if m == 1 else 1.0
                    terms += [(v * s * c, i, j) for c, i, j in _p_terms(1, m - 1, mp, l)]
                    if m != 1:
                        terms += [(-v * c, i, j) for c, i, j in _p_terms(-1, -m + 1, mp, l)]
                else:
                    if m != -1:
                        terms += [(v * c, i, j) for c, i, j in _p_terms(1, m + 1, mp, l)]
                    s = math.sqrt(2.0) if m == -1 else 1.0
                    terms += [(v * s * c, i, j) for c, i, j in _p_terms(-1, -m - 1, mp, l)]
            if w != 0.0:
                if m > 0:
                    terms += [(w * c, i, j) for c, i, j in _p_terms(1, m + 1, mp, l)]
                    terms += [(w * c, i, j) for c, i, j in _p_terms(-1, -m - 1, mp, l)]
                else:
                    terms += [(w * c, i, j) for c, i, j in _p_terms(1, m - 1, mp, l)]
                    terms += [(-w * c, i, j) for c, i, j in _p_terms(-1, -m + 1, mp, l)]
            o = (m + l) * n + (mp + l)
            for c, i, j in terms:
                out_idx.append(o); rij.append(i); ab.append(j); coef.append(c)
    return out_idx, rij, ab, coef


def _mono_mul(k1, k2):
    a = k1[0] + k2[0]
    b = k1[1] + k2[1]
    t1, m1 = k1[2], k1[3]
    t2, m2 = k2[2], k2[3]
    if t1 == 'c' and t2 == 'c':
        tt = [(0.5, 'c', m1 + m2), (0.5, 'c', abs(m1 - m2))]
    elif t1 == 's' and t2 == 's':
        tt = [(-0.5, 'c', m1 + m2), (0.5, 'c', abs(m1 - m2))]
    else:
        ms, mc = (m1, m2) if t1 == 's' else (m2, m1)
        d = ms - mc
        tt = [(0.5, 's', ms + mc)]
        if d > 0:
            tt.append((0.5, 's', d))
        elif d < 0:
            tt.append((-0.5, 's', -d))
    out = []
    for cf, t, m in tt:
        if t == 's' and m == 0:
            continue
        out.append((cf, (a, b, t, m)))
    return out


def _reduce_st(poly):
    changed = True
    while changed:
        changed = False
        for k in list(poly.keys()):
            a, b, t, m = k
            if b >= 2:
                c = poly.pop(k)
                for cf2, anew in ((1.0, a), (-1.0, a + 2)):
                    k2 = (anew, b - 2, t, m)
                    poly[k2] = poly.get(k2, 0.0) + cf2 * c
                changed = True
    return {k: v for k, v in poly.items() if abs(v) > 1e-12}


def _poly_mul(p1, p2):
    out = defaultdict(float)
    for k1, c1 in p1.items():
        for k2, c2 in p2.items():
            for cf, k in _mono_mul(k1, k2):
                out[k] += c1 * c2 * cf
    return _reduce_st(dict(out))


def _build_symbolic_blocks():
    ct = (1, 0, 'c', 0); st = (0, 1, 'c', 0)
    cp = (0, 0, 'c', 1); sp = (0, 0, 's', 1)
    r1 = [
        {cp: 1.0}, {}, {sp: -1.0},
        {(0, 1, 's', 1): 1.0}, {ct: 1.0}, {(0, 1, 'c', 1): 1.0},
        {(1, 0, 's', 1): 1.0}, {st: -1.0}, {(1, 0, 'c', 1): 1.0},
    ]
    blocks = {1: r1}
    for l in range(2, 5):
        oi, rij, ab, cf = _ru_tables(l)
        n = 2 * l + 1
        cur = [defaultdict(float) for _ in range(n * n)]
        prev = blocks[l - 1]
        for o, i, j, c in zip(oi, rij, ab, cf):
            p = _poly_mul(r1[i], prev[j])
            for k, v in p.items():
                cur[o][k] += c * v
        blocks[l] = [_reduce_st(dict(d)) for d in cur]
    return blocks


def build_tables():
    """Returns (monos, W) with W shape [K, 164] fp32. Output cols are the
    flattened l=1..4 block entries in order (l, r, c)."""
    blocks = _build_symbolic_blocks()
    monos = set()
    for l in range(1, 5):
        for poly in blocks[l]:
            monos |= set(poly.keys())
    monos = sorted(monos)
    midx = {m: i for i, m in enumerate(monos)}
    ncols = sum((2 * l + 1) ** 2 for l in range(1, 5))
    W = np.zeros((len(monos), ncols), dtype=np.float64)
    col = 0
    for l in range(1, 5):
        n = 2 * l + 1
        for o in range(n * n):
            for k, v in blocks[l][o].items():
                W[midx[k], col] = v
            col += 1
    return monos, W.astype(np.float32)


MONOS, W_TABLE = build_tables()
K = len(MONOS)          # 81
NOUT = W_TABLE.shape[1]  # 164
assert K <= 128 and NOUT == 164
# fp16 hi/lo split of W: 3-term split GEMM keeps absmax err ~1e-6 while
# halving PE bytes vs the 2-pass fp32 matmul
_WHI = W_TABLE.astype(np.float16)
_WLO = (W_TABLE - _WHI.astype(np.float32)).astype(np.float16)
W_SPLIT = np.concatenate([_WHI, _WLO], axis=1)  # [K, 2*NOUT] fp16

# ----------------------------------------------------------------------------
# Bass program
# ----------------------------------------------------------------------------
N_TOTAL = 200000
N_CORES = 8
N_PER = N_TOTAL // N_CORES  # 25000
DIM = 25
ROW = DIM * DIM  # 625
BCH = 3          # chunks (of 128 points) per batch
NSLOT = 6        # persistent out-tile rotation depth
_OFF = [0, 1, 4, 9, 16]  # block row/col offset per l


def _feat_program(nc, Feat, XYZ, Fs, scratch_pool, dt):
    """Emit wide ops computing all K monomial columns into Feat
    (layout [128, Fs*K], col k of f-slot f at f*K + k)."""
    import concourse.mybir as mybir
    Alu = mybir.AluOpType

    fv = Feat.rearrange("p (f k) -> p k f", k=K)

    def slot(key):
        return fv[:, MONOS.index(key), :]

    xyzv = XYZ.rearrange("p (f k) -> p k f", k=3)
    x, y, z = xyzv[:, 0, :], xyzv[:, 1, :], xyzv[:, 2, :]

    sc = [scratch_pool.tile([128, Fs], dt, name=f"sc{i}", tag=f"sc{i}")
          for i in range(5)]
    s0, s1, s2, s3, s4 = [t[:] for t in sc]

    act, dve = nc.scalar, nc.vector

    def rsqrt_nr(dst, a, tmp):
        """dst = 1/sqrt(a), ACT-sqrt seed + one Newton step (~1 ulp)."""
        act.sqrt(dst, a)
        dve.reciprocal(dst, dst)            # y0 (rel err ~7e-6 from LUT sqrt)
        dve.tensor_mul(tmp, dst, dst)       # y0^2
        dve.tensor_mul(tmp, a, tmp)         # a*y0^2
        dve.tensor_scalar(tmp, tmp, -0.5, 1.5, Alu.mult, Alu.add)
        dve.tensor_mul(dst, dst, tmp)       # y1 = y0*(1.5 - 0.5*a*y0^2)

    # prologue: ct, st, c1, s1
    act.square(s0, x)
    act.square(s1, y)
    dve.tensor_add(s0, s0, s1)          # rxy2
    act.square(s1, z)
    dve.tensor_add(s1, s0, s1)          # r2
    dve.tensor_scalar_max(s0, s0, 1e-37)
    dve.tensor_scalar_max(s1, s1, 1e-28)
    rsqrt_nr(s2, s1, s3)                # rinv  = 1/max(r, 1e-14)
    rsqrt_nr(s4, s0, s3)                # rxyinv
    dve.tensor_mul(s3, z, s2)           # ct raw
    ct = slot((1, 0, 'c', 0))
    dve.tensor_scalar(ct, s3, -1.0, 1.0, Alu.max, Alu.min)
    st = slot((0, 1, 'c', 0))
    dve.tensor_mul(s0, s0, s4)          # rxy = rxy2 * rxyinv
    dve.tensor_mul(st, s0, s2)          # st = rxy * rinv
    c1 = slot((0, 0, 'c', 1))
    s1t = slot((0, 0, 's', 1))
    dve.tensor_mul(c1, x, s4)
    dve.tensor_mul(s1t, y, s4)
    # chebyshev trig: c2=2c1^2-1, s2=2s1c1, cm=2c1*c(m-1)-c(m-2), ...
    trig = {('c', 1): c1, ('s', 1): s1t}
    trig[('c', 0)] = None  # constant one
    for m in range(2, 5):
        cm = slot((0, 0, 'c', m))
        sm = slot((0, 0, 's', m))
        if m == 2:
            dve.scalar_tensor_tensor(s3, c1, 2.0, c1, Alu.mult, Alu.mult)
            dve.tensor_scalar_add(cm, s3, -1.0)
            dve.scalar_tensor_tensor(sm, s1t, 2.0, c1, Alu.mult, Alu.mult)
        else:
            dve.scalar_tensor_tensor(s3, trig[('c', m - 1)], 2.0, c1, Alu.mult, Alu.mult)
            dve.tensor_sub(cm, s3, trig[('c', m - 2)])
            dve.scalar_tensor_tensor(s3, trig[('s', m - 1)], 2.0, c1, Alu.mult, Alu.mult)
            dve.tensor_sub(sm, s3, trig[('s', m - 2)])
        trig[('c', m)] = cm
        trig[('s', m)] = sm
    # ct powers (present as monomials (a,0,'c',0))
    ctp = {1: ct}
    for a in range(2, 5):
        key = (a, 0, 'c', 0)
        dst = slot(key) if key in MONOS else None
        assert dst is not None, key
        h = a // 2
        dve.tensor_mul(dst, ctp[h], ctp[a - h])
        ctp[a] = dst
    # constant-one column
    kone = MONOS.index((0, 0, 'c', 0))
    nc.vector.memset(fv[:, kone, :], 1.0)
    # st*trig columns (0,1,t,m)
    stt = {('c', 0): st}
    for (t, m) in [(t, m) for t in 'cs' for m in range(1, 5)]:
        key = (0, 1, t, m)
        if key in MONOS:
            dve.tensor_mul(slot(key), st, trig[(t, m)])
            stt[(t, m)] = slot(key)
    # remaining monomials: a>=1 times trig or st*trig
    done = {(0, 0, 'c', 0), (1, 0, 'c', 0), (0, 1, 'c', 0)}
    done |= {(0, 0, t, m) for t in 'cs' for m in range(1, 5)}
    done |= {(a, 0, 'c', 0) for a in range(2, 5)}
    done |= {(0, 1, t, m) for t in 'cs' for m in range(1, 5)}
    for key in MONOS:
        if key in done:
            continue
        a, b, t, m = key
        assert a >= 1, key
        base = stt[(t, m)] if b else trig[(t, m)]
        assert base is not None
        dve.tensor_mul(slot(key), ctp[a], base)


def build_program(n_pad, hw=True):
    """n_pad must be a multiple of 128 (host pads input/slices output).
    hw=False skips the walrus wait-limit postprocessing (CoreSim can't
    execute the synthetic NOPs)."""
    import concourse.bass as bass
    import concourse.tile as tile
    import concourse.mybir as mybir
    from concourse.masks import make_identity

    dt = mybir.dt.float32
    assert n_pad % 128 == 0
    Fs = n_pad // 128                  # f-slots (all full)

    nc = bass.Bass()
    f16 = mybir.dt.float16
    xyz_d = nc.declare_dram_parameter("xyz", [n_pad, 3], dt, isOutput=False)
    wc_d = nc.declare_dram_parameter("wc", [K, 2 * NOUT], f16, isOutput=False)
    out_d = nc.declare_dram_parameter("out", [n_pad, ROW], dt, isOutput=True)

    join_sems = {}
    with tile.TileContext(nc) as tc, ExitStack() as ctx:
        join_sems['DVE'] = nc.alloc_semaphore("join_dve")
        join_sems['Activation'] = nc.alloc_semaphore("join_act")
        consts = ctx.enter_context(tc.tile_pool(name="consts", bufs=1))
        scratch = ctx.enter_context(tc.tile_pool(name="scratch", bufs=1))
        featb_pool = ctx.enter_context(tc.tile_pool(name="featb", bufs=4))
        psum_t = ctx.enter_context(tc.tile_pool(name="psumT", bufs=3, space="PSUM"))
        psum_o = ctx.enter_context(tc.tile_pool(name="psumO", bufs=3, space="PSUM"))

        ident = consts.tile([128, 128], dt, tag="ident")
        make_identity(nc, ident[:])
        wc = consts.tile([K, 2 * NOUT], f16, tag="wc")
        nc.sync.dma_start(wc[:], wc_d[:, :])

        XYZ = consts.tile([128, Fs * 3], dt, tag="xyzt")
        nc.sync.dma_start(
            XYZ[:].rearrange("p (f k) -> p f k", k=3),
            xyz_d[:, :].rearrange("(f p) k -> p f k", p=128))

        Feat = consts.tile([128, Fs * K], dt, tag="feat")
        _feat_program(nc, Feat[:], XYZ[:], Fs, scratch, dt)

        # persistent out tiles: zeros + l0 ones written once.  Each slot is
        # written by exactly one compute engine (DVE for even slots, ACT for
        # odd) so downstream instructions never need >2 semaphore waits.
        outs = []
        for i in range(NSLOT):
            t = consts.tile([128, BCH * ROW], dt, name=f"out{i}", tag=f"out{i}")
            v = t[:].rearrange("p (c r q) -> p c r q", r=DIM, q=DIM)
            ones = v[:, :, 0:1, 0:1]
            if i % 2 == 0:
                nc.vector.memset(t[:], 0.0)
                nc.vector.memset(ones, 1.0)
            else:
                nc.scalar.memzero(t[:])
                nc.scalar.add(ones, ones, 1.0)
            outs.append(t)

        featv = Feat[:].rearrange("p (f k) -> p f k", k=K)
        nbatch = (Fs + BCH - 1) // BCH
        for bi in range(nbatch):
            f0 = bi * BCH
            nch = min(BCH, Fs - f0)
            pt = psum_t.tile([K, 128 * BCH], dt, tag="pt")
            for c in range(nch):
                nc.tensor.transpose(pt[:, 128 * c:128 * (c + 1)],
                                    featv[:, f0 + c, :], ident[:])
            fhi = featb_pool.tile([K, 128 * BCH], f16, tag="fhi")
            flo = featb_pool.tile([K, 128 * BCH], f16, tag="flo")
            sl = slice(0, 128 * nch)
            if bi % 2 == 0:
                nc.vector.tensor_copy(fhi[:, sl], pt[:, sl])
            else:
                nc.scalar.copy(fhi[:, sl], pt[:, sl])
            nc.vector.scalar_tensor_tensor(
                flo[:, sl], pt[:, sl], 1.0, fhi[:, sl],
                mybir.AluOpType.mult, mybir.AluOpType.subtract)
            po = psum_o.tile([128, NOUT * BCH], dt, tag="po")
            whi, wlo = wc[:, 0:NOUT], wc[:, NOUT:2 * NOUT]
            for c in range(nch):
                cs = slice(128 * c, 128 * (c + 1))
                os_ = slice(NOUT * c, NOUT * (c + 1))
                nc.tensor.matmul(po[:, os_], fhi[:, cs], whi,
                                 start=True, stop=False)
                nc.tensor.matmul(po[:, os_], fhi[:, cs], wlo,
                                 start=False, stop=False)
                nc.tensor.matmul(po[:, os_], flo[:, cs], whi,
                                 start=False, stop=True)
            # scatter into out tile
            ot = outs[bi % NSLOT]
            ov = ot[:].rearrange("p (c r q) -> p c r q", r=DIM, q=DIM)
            pov = po[:].rearrange("m (c e) -> m c e", c=BCH)
            base = 0
            for l in range(1, 5):
                n = 2 * l + 1
                off = _OFF[l]
                src = pov[:, 0:nch, base:base + n * n].rearrange(
                    "m c (r q) -> m c r q", r=n)
                dst = ov[:, 0:nch, off:off + n, off:off + n]
                if bi % 2 == 0:
                    nc.vector.tensor_copy(dst, src)
                else:
                    nc.scalar.copy(dst, src)
                base += n * n
            # DMA out
            rows0 = f0 * 128
            nc.sync.dma_start(
                out_d[rows0:rows0 + 128 * nch, :].rearrange(
                    "(c p) e -> p c e", p=128),
                ot[:, 0:nch * ROW].rearrange("p (c e) -> p c e", e=ROW))
    nc.finalize()
    if hw:
        fix_dma_waits(nc, join_sems)
        split_sync_waits(nc)
    return nc


_ENG_PREFIX = {'DVE': 'DVE', 'Activation': 'Activation', 'Pool': 'Pool',
               'PE': 'PE', 'SP': 'SP'}


def fix_dma_waits(nc, join_sems):
    """DMA descriptors here accept a single wait.  For DMAs with >1 wait,
    host a NOP chain on the engine whose own semaphore is among the waits
    (queue order makes that wait implicit), absorb the remaining waits there,
    and signal a dedicated join semaphore the DMA waits on instead."""
    import concourse.mybir as mybir
    ET = mybir.EngineType
    eng_map = {'DVE': ET.DVE, 'Activation': ET.Activation, 'Pool': ET.Pool,
               'PE': ET.PE, 'SP': ET.SP}
    counters = {e: 0 for e in join_sems}
    nfix = 0
    for f in nc.m.functions:
        for bb in f.blocks:
            insl = bb.instructions
            i = 0
            while i < len(insl):
                ins = insl[i]
                si = ins.sync_info
                if (ins.opcode in ('DMACopy', 'DMA') and si is not None
                        and len(si.on_wait) > 1):
                    waits = list(si.on_wait)
                    host = None
                    rest = []
                    for w in waits:
                        pfx = w.ant_name.split('_')[0]
                        if host is None and pfx in eng_map and pfx in join_sems:
                            host = pfx  # implied by queue order; dropped
                        else:
                            rest.append(w)
                    assert host is not None, \
                        f"DMA {ins.name} waits {[w.ant_name for w in waits]}"
                    js = join_sems[host]
                    for j, w in enumerate(rest):
                        nop = mybir.InstNoOp(name=f"{ins.name}-dj{j}")
                        nop.engine = eng_map[host]
                        upd = []
                        if j == len(rest) - 1:
                            counters[host] += 1
                            upd = [mybir.SyncUpdate(
                                ant_name=js.name, id=js.num,
                                sync_type='semaphore', update_mode='sem-inc',
                                update_value=1)]
                        nop.sync_info = mybir.SyncInfo(on_wait=[w], on_update=upd)
                        insl.insert(i, nop)
                        i += 1
                    ins.sync_info = mybir.SyncInfo(
                        on_wait=[mybir.SyncWait(
                            ant_name=js.name, id=js.num, sync_type='semaphore',
                            wait_mode='sem-ge-imm', wait_value=counters[host])],
                        on_update=list(si.on_update))
                    nfix += 1
                i += 1
    return nfix


def split_sync_waits(nc, keep=1):
    """The walrus build here accepts at most one embedded semaphore wait per
    engine instruction: hoist extra waits onto single-wait NOPs in front (same
    engine queue, so in-order semantics are identical)."""
    import concourse.mybir as mybir
    nsplit = 0
    for f in nc.m.functions:
        for bb in f.blocks:
            insl = bb.instructions
            i = 0
            while i < len(insl):
                ins = insl[i]
                si = ins.sync_info
                if (si is not None and len(si.on_wait) > keep
                        and ins.opcode not in ('DMACopy', 'DMA')):
                    waits = list(si.on_wait)
                    head, tail = waits[:-keep], waits[-keep:]
                    for j, w in enumerate(head):
                        nop = mybir.InstNoOp(name=f"{ins.name}-sw{j}")
                        nop.engine = ins.engine
                        nop.sync_info = mybir.SyncInfo(on_wait=[w], on_update=[])
                        insl.insert(i, nop)
                        i += 1
                        nsplit += 1
                    ins.sync_info = mybir.SyncInfo(
                        on_wait=tail, on_update=list(si.on_update))
                i += 1
    return nsplit


# ----------------------------------------------------------------------------
# Entry point
# ----------------------------------------------------------------------------
_CACHE = {}


def kernel(xyz: np.ndarray) -> np.ndarray:
    from concourse.bass_utils import run_bass_kernel_spmd

    assert xyz.shape == (N_TOTAL, 3)
    n_pad = ((N_PER + 127) // 128) * 128
    if "nc" not in _CACHE:
        _CACHE["nc"] = build_program(n_pad)
    nc = _CACHE["nc"]

    xyz = np.ascontiguousarray(xyz, dtype=np.float32)
    in_maps = []
    for c in range(N_CORES):
        shard = xyz[c * N_PER:(c + 1) * N_PER]
        if n_pad != N_PER:
            shard = np.concatenate(
                [shard, np.repeat(shard[-1:], n_pad - N_PER, axis=0)], axis=0)
        in_maps.append({"xyz": np.ascontiguousarray(shard), "wc": W_SPLIT})
    res = run_bass_kernel_spmd(nc, in_maps, list(range(N_CORES)))
    outs = [r["out"][:N_PER] for r in res.results]
    full = np.concatenate(outs, axis=0).reshape(N_TOTAL, DIM, DIM)
    return full


if __name__ == "__main__":
    rng = np.random.default_rng(0)
    pts = rng.normal(size=(N_TOTAL, 3)).astype(np.float32)
    out = kernel(pts)
    print(out.shape, out.dtype)
